# revision 10
# baseline (speedup 1.0000x reference)
"""Trainium2 Bass kernel for nn_AdvancedNODE (decision forest, eval mode).

Reference computation:
    fidx = argmax(feature_weights, -1)            # [T, D]
    fv   = x[:, fidx]                             # [B, T, D]
    bits = fv > thresholds                        # [B, T, D]
    dec  = sum_d bits * 2^(D-1-d)                 # [B, T]
    out  = mean_t responses[t, dec[b, t]]         # [B, C]

Strategy (data-parallel over batch, 8 cores, B_loc = B/8 = 2048 per core):

  * Feature phase: x[:, fidx] is a re-layout with host-known indices, so
    the host pre-gathers xg[tau, h, d, b] = x^T[fidx[128h+tau, d], b] and
    the kernel streams it with plain DMA (2MB slices).  DVE computes
    scaled bits (fused is_gt * 2^(7-d), bf16) and a pairwise add tree
    accumulates dec[tau, h, b] in bf16 (exact: dec <= 255).

  * Leaf phase, split two ways:
      - PE route (tree PAIRS): dec rows bounced to DRAM as bf16 and
        re-loaded broadcast across partitions, two trees per DMA
        ([128, 2, 2048]); one DVE is_equal per (pair, l-half) builds
        ohT[l, 2*2048] for both trees at once; PE accumulates
        lhsT=responses[t][l,c] (bf16) into PSUM acc2[16, 2048].
        Emission keeps the PE stream gapless (full 2.4GHz: ~216ns per
        512-col matmul) by running a DVE runway of early h0 pairs
        before phase-A h1 occupies the DVE queue.
      - Q7 route (8 trees per ap_gather call): dec re-wrapped via a
        DRAM bounce into per-Q7-core index lists; ap_gather pulls
        responses[tree, dec] rows; ACT casts results to bf16; at the
        END of the PE stream, sigma-permuted fold matmuls
        (lhsT = wsum) sum each gather into acc2.

  * Finalize: ACT scales acc2 by 1/T and stores out^T [C, B_loc].
"""

import numpy as np

B, F = 16384, 512
T, D, C = 256, 8, 16
L = 1 << D
NCORES = 8
BLOC = B // NCORES          # 2048

NQ7M = (2, 1)               # Q7 m-threshold per h -> 3 gathers, 24 trees
NG = NQ7M[0] + NQ7M[1]
RUNWAY_PAIRS = 14           # h0 pairs emitted on DVE before phase-A h1

_CACHE = {}


def _pe_trees_h(h):
    return [(16 * a + m) for a in range(8) for m in range(NQ7M[h], 16)]


def _pe_trees():
    return [128 * h + tau for h in (0, 1) for tau in _pe_trees_h(h)]


def _build_bass():
    from concourse import bacc, mybir, library_config
    from concourse.tile import TileContext

    f32 = mybir.dt.float32
    bf16 = mybir.dt.bfloat16
    i16 = mybir.dt.int16
    Alu = mybir.AluOpType

    NPE = len(_pe_trees())

    nc = bacc.Bacc()

    xg_d = nc.dram_tensor("xg", [128, 2, D, BLOC], f32, kind="ExternalInput")
    thr_d = nc.dram_tensor("thrT3", [128, 2 * D], f32, kind="ExternalInput")
    respq7_d = nc.dram_tensor("resp_q7", [128, NG * L], f32, kind="ExternalInput")
    resppe_d = nc.dram_tensor("resp_pe", [128, NPE * 2 * C], bf16, kind="ExternalInput")
    wsum_d = nc.dram_tensor("wsum", [128, C], f32, kind="ExternalInput")
    out_d = nc.dram_tensor("out", [C, BLOC], f32, kind="ExternalOutput")

    # DRAM bounces
    iw_d = nc.dram_tensor("iwtmp", [8, 16, 2, 16, 128], i16)
    dec16_d = nc.dram_tensor("dec16tmp", [128, 2, BLOC], bf16)

    # PE pair order: h0 pairs then h1 pairs (trees inside one h-half).
    pe_pairs = []
    for h in (0, 1):
        ts = _pe_trees_h(h)
        for i in range(0, len(ts), 2):
            pe_pairs.append((h, ts[i], ts[i + 1]))

    with TileContext(nc) as tc:
        with (
            tc.tile_pool(name="const", bufs=1) as constp,
            tc.tile_pool(name="fvp", bufs=2) as fvp,
            tc.tile_pool(name="sbp", bufs=3) as sbp,
            tc.tile_pool(name="replp", bufs=6) as replp,
            tc.tile_pool(name="ohp", bufs=5) as ohp,
            tc.tile_pool(name="goutp", bufs=2) as goutp,
            tc.tile_pool(name="psum", bufs=1, space="PSUM") as psump,
        ):
            # ---- constants ----
            # load the ap_gather Q7 ucode FIRST so its image DMA isn't
            # starved behind the const/xg DMA flood
            nc.gpsimd.load_library(library_config.ap_gather)
            t_thr = constp.tile([128, 2 * D], f32)
            nc.sync.dma_start(out=t_thr[:], in_=thr_d[:])
            t_rq7 = constp.tile([128, NG, L], f32)
            nc.sync.dma_start(
                out=t_rq7[:], in_=respq7_d[:].rearrange("p (g l) -> p g l", g=NG)
            )
            t_rpe = constp.tile([128, NPE, 2, C], bf16)
            nc.sync.dma_start(
                out=t_rpe[:],
                in_=resppe_d[:].rearrange("p (x u c) -> p x u c", x=NPE, u=2),
            )
            t_wsum = constp.tile([128, C], f32)
            nc.sync.dma_start(out=t_wsum[:], in_=wsum_d[:])
            t_wsum16 = constp.tile([128, C], bf16)
            nc.vector.tensor_copy(out=t_wsum16[:], in_=t_wsum[:])

            t_iota_i = constp.tile([128, 2], mybir.dt.int32)
            nc.gpsimd.iota(t_iota_i[:], pattern=[[128, 2]], base=0,
                           channel_multiplier=1)
            t_iota = constp.tile([128, 2], f32)
            nc.vector.tensor_copy(out=t_iota[:], in_=t_iota_i[:])

            t_dec = [constp.tile([128, BLOC], bf16, name=f"dec{h}",
                                 tag=f"dec{h}") for h in (0, 1)]
            t_decT = [constp.tile([128, BLOC], i16, name=f"decT{h}",
                                  tag=f"decT{h}") for h in (0, 1)]
            t_iw = constp.tile([128, 2, NQ7M[0], 128], i16)
            t_g16 = [constp.tile([128, BLOC], bf16, name=f"g16_{g}",
                                 tag=f"g16_{g}") for g in range(NG)]

            p_acc2 = psump.tile([C, BLOC], f32, space="PSUM", tag="acc2")

            # ---------------- emission helpers ----------------
            def emit_phaseA(h):
                for dp in range(D // 2):
                    t_xs = fvp.tile([128, 2, BLOC], f32, tag="xs")
                    nc.sync.dma_start(
                        out=t_xs[:], in_=xg_d[:, h, 2 * dp:2 * dp + 2, :])
                    sb = [sbp.tile([128, BLOC], bf16, name=f"sb{h}_{dp}_{q_}",
                                   tag="sb") for q_ in range(2)]
                    for ql in range(2):
                        d_ = 2 * dp + ql
                        nc.vector.tensor_scalar(
                            out=sb[ql][:], in0=t_xs[:, ql, :],
                            scalar1=t_thr[:, D * h + d_:D * h + d_ + 1],
                            scalar2=float(1 << (D - 1 - d_)),
                            op0=Alu.is_gt, op1=Alu.mult,
                        )
                    if dp == 0:
                        nc.vector.tensor_tensor(
                            out=t_dec[h][:], in0=sb[0][:], in1=sb[1][:],
                            op=Alu.add)
                    else:
                        nc.vector.tensor_tensor(
                            out=sb[1][:], in0=sb[0][:], in1=sb[1][:],
                            op=Alu.add)
                        nc.vector.tensor_tensor(
                            out=t_dec[h][:], in0=t_dec[h][:], in1=sb[1][:],
                            op=Alu.add)

            def emit_decwrap(h):
                nc.scalar.copy(out=t_decT[h][:], in_=t_dec[h][:])
                nc.sync.dma_start(out=dec16_d[:, h, :], in_=t_dec[h][:])
                mq = NQ7M[h]
                for a in range(8):
                    nc.sync.dma_start(
                        out=iw_d[a, :, h, 0:mq, :].rearrange("j m s -> m j s"),
                        in_=t_decT[h][16 * a:16 * a + mq, :].rearrange(
                            "p (j s) -> p j s", j=16),
                    )
                nc.sync.dma_start(
                    out=t_iw[:, h, 0:mq, :],
                    in_=iw_d[:, :, h, 0:mq, :].rearrange("a j m s -> (a j) m s"),
                )

            gouts = {}

            def emit_q7(h):
                for m in range(NQ7M[h]):
                    g = m if h == 0 else NQ7M[0] + m
                    t_gout = goutp.tile([128, BLOC], f32, tag="gout")
                    nc.gpsimd.ap_gather(
                        t_gout[:], t_rq7[:, g, :], t_iw[:, h, m, :],
                        channels=128, num_elems=L, d=1, num_idxs=BLOC,
                    )
                    gouts[g] = t_gout

            def emit_q7_casts(gs):
                # ACT casts, emitted after both decT copies so the ACT
                # queue never blocks wrap work behind a gather wait
                for g in gs:
                    nc.vector.tensor_copy(out=t_g16[g][:], in_=gouts[g][:])

            xi_map = {t: i for i, t in enumerate(_pe_trees())}
            first_mm = [True]

            bq = [0]

            def emit_pe_pair_dve(h, tau0, tau1):
                """bcast pair + one is_equal per l-half covering both trees."""
                t_repl = replp.tile([128, 2, BLOC], bf16, tag="repl")
                eng = nc.scalar if bq[0] % 2 == 0 else nc.sync
                bq[0] += 1
                eng.dma_start(
                    out=t_repl[:, 0, :],
                    in_=dec16_d[tau0:tau0 + 1, h, :].to_broadcast([128, BLOC]))
                eng.dma_start(
                    out=t_repl[:, 1, :],
                    in_=dec16_d[tau1:tau1 + 1, h, :].to_broadcast([128, BLOC]))
                ohs = []
                for lh in range(2):
                    t_oh = ohp.tile([128, 2, BLOC], bf16, tag="oh")
                    nc.vector.tensor_scalar(
                        out=t_oh[:], in0=t_repl[:],
                        scalar1=t_iota[:, lh:lh + 1], scalar2=None,
                        op0=Alu.is_equal,
                    )
                    ohs.append(t_oh)
                return ohs

            def emit_pe_pair_mm(h, tau0, tau1, ohs):
                for k, tau in enumerate((tau0, tau1)):
                    xi = xi_map[128 * h + tau]
                    for lh in range(2):
                        # start=True must cover ALL four psum banks once
                        first = first_mm[0] and k == 0 and lh == 0
                        for n in range(BLOC // 512):
                            nc.tensor.matmul(
                                out=p_acc2[:, n * 512:(n + 1) * 512],
                                lhsT=t_rpe[:, xi, lh, :],
                                rhs=ohs[lh][:, k, n * 512:(n + 1) * 512],
                                start=first, stop=False,
                            )
                    first_mm[0] = False

            # ---------------- emission order ----------------
            h0_pairs = [p for p in pe_pairs if p[0] == 0]
            h1_pairs = [p for p in pe_pairs if p[0] == 1]

            emit_phaseA(0)
            emit_decwrap(0)
            emit_q7(0)
            # DVE runway: early h0 pairs queued on DVE before phase-A h1
            pend = []
            for p in h0_pairs[:RUNWAY_PAIRS]:
                ohs = emit_pe_pair_dve(*p)
                pend.append((p, ohs))
                # interleave matmuls immediately so PE starts ASAP
                emit_pe_pair_mm(*p, ohs)
            emit_phaseA(1)
            emit_decwrap(1)
            emit_q7(1)
            for p in h0_pairs[RUNWAY_PAIRS:] + h1_pairs:
                ohs = emit_pe_pair_dve(*p)
                emit_pe_pair_mm(*p, ohs)
            # casts LAST on DVE so they never block the is_equal stream
            emit_q7_casts(range(NG))

            # sigma-permuted fold of each Q7 gather into acc2:
            # fold output column k takes gather column 16*(k%128)+k//128
            for g in range(NG):
                gp = t_g16[g][:].rearrange("p (s j) -> p j s", j=16)
                for n in range(BLOC // 512):
                    nc.tensor.matmul(
                        out=p_acc2[:, n * 512:(n + 1) * 512],
                        lhsT=t_wsum16[:],
                        rhs=gp[:, n * 4:(n + 1) * 4, :],
                        start=False, stop=(g == NG - 1),
                    )

            # ---- finalize: scale by 1/T, store out^T ----
            t_out = constp.tile([C, BLOC], f32, tag="outt")
            nc.scalar.mul(out=t_out[:], in_=p_acc2[:], mul=1.0 / T)
            nc.sync.dma_start(out=out_d[:], in_=t_out[:])

    nc.finalize()
    return nc


def _host_prep(feature_weights, thresholds, responses):
    import ml_dtypes

    fidx = np.argmax(feature_weights, axis=-1)          # [T, D]

    # thrT3[tau, D*h + d] = thresholds[128h+tau, d]
    thrT3 = np.ascontiguousarray(
        thresholds.reshape(2, 128, D).transpose(1, 0, 2).reshape(128, 2 * D)
    ).astype(np.float32)

    # Q7 tables: g=(h,m): core a handles tree 128h + 16a + m, m < NQ7M[h]
    resp_q7 = np.empty((128, NG, L), np.float32)
    g = 0
    for h in (0, 1):
        for m in range(NQ7M[h]):
            for a in range(8):
                tree = 128 * h + 16 * a + m
                for c in range(C):
                    resp_q7[16 * a + c, g] = responses[tree, :, c]
            g += 1
    resp_q7 = resp_q7.reshape(128, NG * L)

    # PE tables: resp_pe[lp, x, lh, c] = responses[tree_x, 128*lh + lp, c]
    trees = _pe_trees()
    rp = responses[trees].reshape(len(trees), 2, 128, C)       # [x, lh, lp, c]
    resp_pe = np.ascontiguousarray(rp.transpose(2, 0, 1, 3)).reshape(
        128, len(trees) * 2 * C).astype(ml_dtypes.bfloat16)

    wsum = np.zeros((128, C), np.float32)
    wsum[np.arange(128), np.arange(128) % C] = 1.0
    return fidx, thrT3, resp_q7, resp_pe, wsum


def kernel(x, feature_weights, thresholds, responses):
    x = np.asarray(x, dtype=np.float32)
    feature_weights = np.asarray(feature_weights, dtype=np.float32)
    thresholds = np.asarray(thresholds, dtype=np.float32)
    responses = np.asarray(responses, dtype=np.float32)

    fidx, thrT3, resp_q7, resp_pe, wsum = _host_prep(
        feature_weights, thresholds, responses
    )
    fidx_r = fidx.reshape(2, 128, D)                    # [h, tau, d]

    if "nc" not in _CACHE:
        _CACHE["nc"] = _build_bass()
    nc = _CACHE["nc"]

    in_maps = []
    for core in range(NCORES):
        xt = np.ascontiguousarray(x[core * BLOC:(core + 1) * BLOC].T)
        # xg[tau, h, d, b] = xt[fidx[128h+tau, d], b]
        xg = np.ascontiguousarray(xt[fidx_r].transpose(1, 0, 2, 3))
        in_maps.append({
            "xg": xg,
            "thrT3": thrT3,
            "resp_q7": resp_q7,
            "resp_pe": resp_pe,
            "wsum": wsum,
        })

    from concourse.bass_utils import run_bass_kernel_spmd
    import os
    kw = {}
    if os.environ.get("KERNEL_TRACE"):
        try:
            import sys, types
            import antenv  # noqa
            if "antenv.axon_hooks" not in sys.modules:
                from trn_agent_boot.trn_boot import _ntff_profile_via_ctypes
                _h = _ntff_profile_via_ctypes("/opt/axon/libaxon_pjrt.so")
                _mod = types.ModuleType("antenv.axon_hooks")
                _mod.get_axon_ntff_profile_hook = lambda: _h
                _mod.set_axon_ntff_profile_hook = lambda h: None
                sys.modules["antenv.axon_hooks"] = _mod
            kw = dict(trace=True, trace_cores=[0])
        except Exception:
            pass
    res = run_bass_kernel_spmd(nc, in_maps, list(range(NCORES)), **kw)
    _CACHE["last_exec_time_ns"] = getattr(res, "exec_time_ns", None)
    _CACHE["last_trace"] = getattr(res, "instructions_and_trace", None)
    _CACHE["last_results"] = res.results

    out = np.empty((B, C), np.float32)
    for core in range(NCORES):
        out[core * BLOC:(core + 1) * BLOC] = res.results[core]["out"].T
    return out


# revision 12
# speedup vs baseline: 1.4607x; 1.4607x over previous
"""Trainium2 Bass kernel for nn_AdvancedNODE (decision forest, eval mode).

Reference computation:
    fidx = argmax(feature_weights, -1)            # [T, D]
    fv   = x[:, fidx]                             # [B, T, D]
    bits = fv > thresholds                        # [B, T, D]
    dec  = sum_d bits * 2^(D-1-d)                 # [B, T]
    out  = mean_t responses[t, dec[b, t]]         # [B, C]

Strategy (data-parallel over batch, 8 cores, B_loc = B/8 = 2048 per core):

  * Feature phase: x[:, fidx] is a re-layout with host-known indices, so
    the host pre-gathers xg[tau, h, d, b] = x^T[fidx[128h+tau, d], b] and
    the kernel streams it with plain DMA (2MB slices).  DVE computes
    scaled bits (fused is_gt * 2^(7-d), bf16) and a pairwise add tree
    accumulates dec[tau, h, b] in bf16 (exact: dec <= 255).

  * Leaf phase, split two ways:
      - PE route (tree PAIRS): dec rows bounced to DRAM as bf16 and
        re-loaded broadcast across partitions, two trees per DMA
        ([128, 2, 2048]); one DVE is_equal per (pair, l-half) builds
        ohT[l, 2*2048] for both trees at once; PE accumulates
        lhsT=responses[t][l,c] (bf16) into PSUM acc2[16, 2048].
        Emission keeps the PE stream gapless (full 2.4GHz: ~216ns per
        512-col matmul) by running a DVE runway of early h0 pairs
        before phase-A h1 occupies the DVE queue.
      - Q7 route (8 trees per ap_gather call): dec re-wrapped via a
        DRAM bounce into per-Q7-core index lists; ap_gather pulls
        responses[tree, dec] rows; ACT casts results to bf16; at the
        END of the PE stream, sigma-permuted fold matmuls
        (lhsT = wsum) sum each gather into acc2.

  * Finalize: ACT scales acc2 by 1/T and stores out^T [C, B_loc].
"""

import numpy as np

B, F = 16384, 512
T, D, C = 256, 8, 16
L = 1 << D
NCORES = 8
BLOC = B // NCORES          # 2048

NQ7M = (2, 1)               # Q7 m-threshold per h -> 3 gathers, 24 trees
NG = NQ7M[0] + NQ7M[1]
RUNWAY_PAIRS = 14           # h0 pairs emitted on DVE before phase-A h1

_CACHE = {}


def _pe_trees_h(h):
    return [(16 * a + m) for a in range(8) for m in range(NQ7M[h], 16)]


def _pe_trees():
    return [128 * h + tau for h in (0, 1) for tau in _pe_trees_h(h)]


def _build_bass():
    from concourse import bacc, mybir, library_config
    from concourse.tile import TileContext

    f32 = mybir.dt.float32
    bf16 = mybir.dt.bfloat16
    i16 = mybir.dt.int16
    Alu = mybir.AluOpType

    NPE = len(_pe_trees())

    nc = bacc.Bacc()

    xg_d = nc.dram_tensor("xg", [128, 2, D, BLOC], f32, kind="ExternalInput")
    thr_d = nc.dram_tensor("thrT3", [128, 2 * D], f32, kind="ExternalInput")
    respq7_d = nc.dram_tensor("resp_q7", [128, NG * L], f32, kind="ExternalInput")
    resppe_d = nc.dram_tensor("resp_pe", [128, NPE * 2 * C], bf16, kind="ExternalInput")
    wsum_d = nc.dram_tensor("wsum", [128, C], f32, kind="ExternalInput")
    out_d = nc.dram_tensor("out", [C, BLOC], f32, kind="ExternalOutput")

    # DRAM bounces
    iw_d = nc.dram_tensor("iwtmp", [8, 16, 2, 16, 128], i16)
    dec16_d = nc.dram_tensor("dec16tmp", [128, 2, BLOC], bf16)

    # PE pair order: h0 pairs then h1 pairs (trees inside one h-half).
    pe_pairs = []
    for h in (0, 1):
        ts = _pe_trees_h(h)
        for i in range(0, len(ts), 2):
            pe_pairs.append((h, ts[i], ts[i + 1]))

    with TileContext(nc) as tc:
        with (
            tc.tile_pool(name="const", bufs=1) as constp,
            tc.tile_pool(name="fvp", bufs=2) as fvp,
            tc.tile_pool(name="sbp", bufs=3) as sbp,
            tc.tile_pool(name="replp", bufs=6) as replp,
            tc.tile_pool(name="ohp", bufs=5) as ohp,
            tc.tile_pool(name="goutp", bufs=3) as goutp,
            tc.tile_pool(name="psum", bufs=1, space="PSUM") as psump,
        ):
            # ---- constants ----
            # load the ap_gather Q7 ucode FIRST so its image DMA isn't
            # starved behind the const/xg DMA flood
            nc.gpsimd.load_library(library_config.ap_gather)
            t_thr = constp.tile([128, 2 * D], f32)
            nc.sync.dma_start(out=t_thr[:], in_=thr_d[:])
            t_rq7 = constp.tile([128, NG, L], f32)
            nc.sync.dma_start(
                out=t_rq7[:], in_=respq7_d[:].rearrange("p (g l) -> p g l", g=NG)
            )
            t_rpe = constp.tile([128, NPE, 2, C], bf16)
            nc.sync.dma_start(
                out=t_rpe[:],
                in_=resppe_d[:].rearrange("p (x u c) -> p x u c", x=NPE, u=2),
            )
            t_wsum = constp.tile([128, C], f32)
            nc.sync.dma_start(out=t_wsum[:], in_=wsum_d[:])

            t_iota_i = constp.tile([128, 2], mybir.dt.int32)
            nc.gpsimd.iota(t_iota_i[:], pattern=[[128, 2]], base=0,
                           channel_multiplier=1)
            t_iota = constp.tile([128, 2], f32)
            nc.vector.tensor_copy(out=t_iota[:], in_=t_iota_i[:])

            t_dec = [constp.tile([128, BLOC], bf16, name=f"dec{h}",
                                 tag=f"dec{h}") for h in (0, 1)]
            t_decT = [constp.tile([128, BLOC], i16, name=f"decT{h}",
                                  tag=f"decT{h}") for h in (0, 1)]
            t_iw = constp.tile([128, 2, NQ7M[0], 128], i16)

            p_acc2 = psump.tile([C, BLOC], f32, space="PSUM", tag="acc2")

            # ---------------- emission helpers ----------------
            def emit_phaseA(h):
                for dp in range(D // 2):
                    t_xs = fvp.tile([128, 2, BLOC], f32, tag="xs")
                    nc.sync.dma_start(
                        out=t_xs[:], in_=xg_d[:, h, 2 * dp:2 * dp + 2, :])
                    sb = [sbp.tile([128, BLOC], bf16, name=f"sb{h}_{dp}_{q_}",
                                   tag="sb") for q_ in range(2)]
                    for ql in range(2):
                        d_ = 2 * dp + ql
                        nc.vector.tensor_scalar(
                            out=sb[ql][:], in0=t_xs[:, ql, :],
                            scalar1=t_thr[:, D * h + d_:D * h + d_ + 1],
                            scalar2=float(1 << (D - 1 - d_)),
                            op0=Alu.is_gt, op1=Alu.mult,
                        )
                    if dp == 0:
                        nc.vector.tensor_tensor(
                            out=t_dec[h][:], in0=sb[0][:], in1=sb[1][:],
                            op=Alu.add)
                    else:
                        nc.vector.tensor_tensor(
                            out=sb[1][:], in0=sb[0][:], in1=sb[1][:],
                            op=Alu.add)
                        nc.vector.tensor_tensor(
                            out=t_dec[h][:], in0=t_dec[h][:], in1=sb[1][:],
                            op=Alu.add)

            def emit_decwrap(h):
                nc.scalar.copy(out=t_decT[h][:], in_=t_dec[h][:])
                nc.sync.dma_start(out=dec16_d[:, h, :], in_=t_dec[h][:])
                mq = NQ7M[h]
                for a in range(8):
                    nc.sync.dma_start(
                        out=iw_d[a, :, h, 0:mq, :].rearrange("j m s -> m j s"),
                        in_=t_decT[h][16 * a:16 * a + mq, :].rearrange(
                            "p (j s) -> p j s", j=16),
                    )
                nc.sync.dma_start(
                    out=t_iw[:, h, 0:mq, :],
                    in_=iw_d[:, :, h, 0:mq, :].rearrange("a j m s -> (a j) m s"),
                )

            gouts = {}

            def emit_q7(h):
                for m in range(NQ7M[h]):
                    g = m if h == 0 else NQ7M[0] + m
                    t_gout = goutp.tile([128, BLOC], f32, tag="gout")
                    nc.gpsimd.ap_gather(
                        t_gout[:], t_rq7[:, g, :], t_iw[:, h, m, :],
                        channels=128, num_elems=L, d=1, num_idxs=BLOC,
                    )
                    gouts[g] = t_gout


            xi_map = {t: i for i, t in enumerate(_pe_trees())}
            first_mm = [True]

            bq = [0]

            def emit_pe_pair_dve(h, tau0, tau1):
                """bcast pair + one is_equal per l-half covering both trees."""
                t_repl = replp.tile([128, 2, BLOC], bf16, tag="repl")
                eng = nc.scalar if bq[0] % 2 == 0 else nc.sync
                bq[0] += 1
                eng.dma_start(
                    out=t_repl[:, 0, :],
                    in_=dec16_d[tau0:tau0 + 1, h, :].to_broadcast([128, BLOC]))
                eng.dma_start(
                    out=t_repl[:, 1, :],
                    in_=dec16_d[tau1:tau1 + 1, h, :].to_broadcast([128, BLOC]))
                ohs = []
                for lh in range(2):
                    t_oh = ohp.tile([128, 2, BLOC], bf16, tag="oh")
                    nc.vector.tensor_scalar(
                        out=t_oh[:], in0=t_repl[:],
                        scalar1=t_iota[:, lh:lh + 1], scalar2=None,
                        op0=Alu.is_equal,
                    )
                    ohs.append(t_oh)
                return ohs

            def emit_pe_pair_mm(h, tau0, tau1, ohs):
                for k, tau in enumerate((tau0, tau1)):
                    xi = xi_map[128 * h + tau]
                    for lh in range(2):
                        # start=True must cover ALL four psum banks once
                        first = first_mm[0] and k == 0 and lh == 0
                        for n in range(BLOC // 512):
                            nc.tensor.matmul(
                                out=p_acc2[:, n * 512:(n + 1) * 512],
                                lhsT=t_rpe[:, xi, lh, :],
                                rhs=ohs[lh][:, k, n * 512:(n + 1) * 512],
                                start=first, stop=False,
                            )
                    first_mm[0] = False

            # ---------------- emission order ----------------
            h0_pairs = [p for p in pe_pairs if p[0] == 0]
            h1_pairs = [p for p in pe_pairs if p[0] == 1]

            emit_phaseA(0)
            emit_decwrap(0)
            emit_q7(0)
            # DVE runway: early h0 pairs queued on DVE before phase-A h1
            pend = []
            for p in h0_pairs[:RUNWAY_PAIRS]:
                ohs = emit_pe_pair_dve(*p)
                pend.append((p, ohs))
                # interleave matmuls immediately so PE starts ASAP
                emit_pe_pair_mm(*p, ohs)
            emit_phaseA(1)
            emit_decwrap(1)
            emit_q7(1)
            for p in h0_pairs[RUNWAY_PAIRS:] + h1_pairs:
                ohs = emit_pe_pair_dve(*p)
                emit_pe_pair_mm(*p, ohs)

            # sigma-permuted fold of each Q7 gather into acc2 (f32r, no
            # casts); tile_wait_until pins them at the schedule end so the
            # Tile scheduler cannot hoist them into the matmul stream
            with tc.tile_wait_until(900.0):
                for g in range(NG):
                    gp = gouts[g][:].rearrange("p (s j) -> p j s", j=16)
                    for n in range(BLOC // 512):
                        nc.tensor.matmul(
                            out=p_acc2[:, n * 512:(n + 1) * 512],
                            lhsT=t_wsum[:],
                            rhs=gp[:, n * 4:(n + 1) * 4, :],
                            start=False, stop=(g == NG - 1),
                        )

            # ---- finalize: scale by 1/T, store out^T ----
            t_out = constp.tile([C, BLOC], f32, tag="outt")
            nc.scalar.mul(out=t_out[:], in_=p_acc2[:], mul=1.0 / T)
            nc.sync.dma_start(out=out_d[:], in_=t_out[:])

    nc.finalize()
    return nc


def _host_prep(feature_weights, thresholds, responses):
    import ml_dtypes

    fidx = np.argmax(feature_weights, axis=-1)          # [T, D]

    # thrT3[tau, D*h + d] = thresholds[128h+tau, d]
    thrT3 = np.ascontiguousarray(
        thresholds.reshape(2, 128, D).transpose(1, 0, 2).reshape(128, 2 * D)
    ).astype(np.float32)

    # Q7 tables: g=(h,m): core a handles tree 128h + 16a + m, m < NQ7M[h]
    resp_q7 = np.empty((128, NG, L), np.float32)
    g = 0
    for h in (0, 1):
        for m in range(NQ7M[h]):
            for a in range(8):
                tree = 128 * h + 16 * a + m
                for c in range(C):
                    resp_q7[16 * a + c, g] = responses[tree, :, c]
            g += 1
    resp_q7 = resp_q7.reshape(128, NG * L)

    # PE tables: resp_pe[lp, x, lh, c] = responses[tree_x, 128*lh + lp, c]
    trees = _pe_trees()
    rp = responses[trees].reshape(len(trees), 2, 128, C)       # [x, lh, lp, c]
    resp_pe = np.ascontiguousarray(rp.transpose(2, 0, 1, 3)).reshape(
        128, len(trees) * 2 * C).astype(ml_dtypes.bfloat16)

    wsum = np.zeros((128, C), np.float32)
    wsum[np.arange(128), np.arange(128) % C] = 1.0
    return fidx, thrT3, resp_q7, resp_pe, wsum


def kernel(x, feature_weights, thresholds, responses):
    x = np.asarray(x, dtype=np.float32)
    feature_weights = np.asarray(feature_weights, dtype=np.float32)
    thresholds = np.asarray(thresholds, dtype=np.float32)
    responses = np.asarray(responses, dtype=np.float32)

    fidx, thrT3, resp_q7, resp_pe, wsum = _host_prep(
        feature_weights, thresholds, responses
    )
    fidx_r = fidx.reshape(2, 128, D)                    # [h, tau, d]

    if "nc" not in _CACHE:
        _CACHE["nc"] = _build_bass()
    nc = _CACHE["nc"]

    in_maps = []
    for core in range(NCORES):
        xt = np.ascontiguousarray(x[core * BLOC:(core + 1) * BLOC].T)
        # xg[tau, h, d, b] = xt[fidx[128h+tau, d], b]
        xg = np.ascontiguousarray(xt[fidx_r].transpose(1, 0, 2, 3))
        in_maps.append({
            "xg": xg,
            "thrT3": thrT3,
            "resp_q7": resp_q7,
            "resp_pe": resp_pe,
            "wsum": wsum,
        })

    from concourse.bass_utils import run_bass_kernel_spmd
    import os
    kw = {}
    if os.environ.get("KERNEL_TRACE"):
        try:
            import sys, types
            import antenv  # noqa
            if "antenv.axon_hooks" not in sys.modules:
                from trn_agent_boot.trn_boot import _ntff_profile_via_ctypes
                _h = _ntff_profile_via_ctypes("/opt/axon/libaxon_pjrt.so")
                _mod = types.ModuleType("antenv.axon_hooks")
                _mod.get_axon_ntff_profile_hook = lambda: _h
                _mod.set_axon_ntff_profile_hook = lambda h: None
                sys.modules["antenv.axon_hooks"] = _mod
            kw = dict(trace=True, trace_cores=[0])
        except Exception:
            pass
    res = run_bass_kernel_spmd(nc, in_maps, list(range(NCORES)), **kw)
    _CACHE["last_exec_time_ns"] = getattr(res, "exec_time_ns", None)
    _CACHE["last_trace"] = getattr(res, "instructions_and_trace", None)
    _CACHE["last_results"] = res.results

    out = np.empty((B, C), np.float32)
    for core in range(NCORES):
        out[core * BLOC:(core + 1) * BLOC] = res.results[core]["out"].T
    return out
